# revision 1
# baseline (speedup 1.0000x reference)
"""Trainium2 Bass kernel: C = triu(A @ B), A/B upper-triangular 4096x4096 fp32.

Strategy (row-parallel over 8 cores, SPMD single program):
  * 32 row-blocks of 128 rows. Core c owns blocks {c, 8+c, 16+c, 24+c}
    ("slot" j = block 8j + c).
  * One uniform schedule for all cores: for column tile q (8 tiles of 512)
    and slot j, accumulate k-tiles k in [8j, 4q+3].  Per-core variation
    lives entirely in the DATA: the host packs A^T tiles per core and
    zero-fills tiles with k < own-block, so padded matmuls contribute
    exact zeros.  Since A and B are both upper-triangular, the lower
    triangle of C comes out exactly 0 - no masking needed.
  * A^T pack (80 tiles of 128x128) is cached in SBUF; B streams once per
    column tile with below-diagonal tiles skipped.
"""

import numpy as np
from contextlib import ExitStack

import concourse.mybir as mybir
import concourse.tile as tile
from concourse import bacc, bass_utils

N = 4096
P = 128
NCORES = 8
NSLOT = 4          # row-block slots per core
NQ = 8             # 512-wide output column tiles
QW = 512
NKT = 32           # 128-wide k tiles
KSTART = [0, 8, 16, 24]            # first k-tile per slot (min over cores)
ANT = [32, 24, 16, 8]              # k-tiles stored per slot
AOFF = [0, 32, 56, 72]             # slot offsets into the A pack
ATOT = 80                          # total packed A tiles per core

# (slot, qtile) pairs the program computes/writes, in emission order
PAIRS = [(j, q) for q in range(NQ) for j in range(NSLOT) if 4 * q + 4 > 8 * j]
NT = len(PAIRS)                    # 20 output tiles of 128x512 per core

# matmul dtype mode: "fp32r" (fast, ~11-bit mantissa), "bf16x3" (hi/lo
# 3-pass split, near-fp32 accuracy), "fp32" (exact, 4x slower PE)
MODE = "fp32r"

# pool buffer counts (double/triple buffering)
BUFS_B = 3
BUFS_O = 4
BUFS_PS = 8

_nc_cache = {}


def build_nc(mode=MODE, rep=1, variant="full"):
    """rep>1 repeats the whole compute (for dispatch-overhead-cancelling
    timing): T_hw ~= (T(rep=R) - T(rep=1)) / (R-1).
    variant: "full" | "nomm" (DMAs only) | "nodma" (matmuls only)."""
    if (mode, rep, variant) in _nc_cache:
        return _nc_cache[(mode, rep, variant)]
    two = 2 if mode == "bf16x3" else 1
    dt_in = {
        "fp32r": mybir.dt.float32r,
        "bf16x3": mybir.dt.bfloat16,
        "fp32": mybir.dt.float32,
    }[mode]

    nc = bacc.Bacc("TRN2", target_bir_lowering=False, debug=False,
                   num_devices=NCORES)
    # partition-major packed layouts (see pack_inputs): per-partition data is
    # contiguous so every DMA is 128 descriptors of large contiguous runs.
    # Apack row = h*P + p(k-within-tile), col = t*P + m  (40KB/partition)
    a_dram = nc.dram_tensor("Apack", [two * P, ATOT * P], dt_in,
                            kind="ExternalInput").ap()
    # B row = (h*NQ + q)*P + p, col = k*QW + n          (8KB runs/partition)
    b_dram = nc.dram_tensor("B", [two * NQ * P, NKT * QW], dt_in,
                            kind="ExternalInput").ap()
    c_dram = nc.dram_tensor("Cout", [NT * P, QW], mybir.dt.float32,
                            kind="ExternalOutput").ap()

    with tile.TileContext(nc) as tc:
        with ExitStack() as ctx:
            apool = ctx.enter_context(tc.tile_pool(name="apool", bufs=1))
            bpool = ctx.enter_context(tc.tile_pool(name="bpool", bufs=BUFS_B))
            opool = ctx.enter_context(tc.tile_pool(name="opool", bufs=BUFS_O))
            pspool = ctx.enter_context(
                tc.tile_pool(name="pspool", bufs=BUFS_PS, space="PSUM"))

            do_bdma = variant in ("full", "nomm", "vbdma")
            do_mm = variant in ("full", "nodma", "vmm")
            do_copy = variant in ("full", "nomm", "nodma", "vcopy")
            do_store = variant in ("full", "nomm", "nodma", "vstore")

            # A load split so early matmuls are gated only by the tiles they
            # read: slot0 k0..7 (feeds q=1/q=0) lands in ~1.5us, the rest
            # overlaps with the B stream.
            a_sb = apool.tile([P, two, ATOT, P], dt_in)
            for t0, t1 in [(0, 8), (8, 32), (32, ATOT)]:
                for h in range(two):
                    nc.sync.dma_start(
                        a_sb[:, h, t0:t1, :],
                        a_dram[h * P:(h + 1) * P, t0 * P:t1 * P].rearrange(
                            "p (t m) -> p t m", m=P))

            # micro variants: per rep emit n tiny ops, skip the main loop
            micro = variant.startswith("vd") or variant in ("vgps8", "vdve8")
            if micro:
                n_ops = (8 if variant in ("vgps8", "vdve8")
                         else int(variant[2:]))
                mpool = ctx.enter_context(tc.tile_pool(name="mp", bufs=16))
                for r in range(rep):
                    for i in range(n_ops):
                        mt = mpool.tile([P, QW], mybir.dt.float32, tag="mt",
                                        name=f"mt_{r}_{i}")
                        if variant == "vdve8":
                            src = a_sb[:, 0, 4 * i:4 * i + 4, :]
                            if dt_in == mybir.dt.float32r:
                                src = src.bitcast(mybir.dt.float32)
                            nc.vector.tensor_copy(
                                mt[:].rearrange("p (a b) -> p a b", a=4),
                                src)
                        elif variant == "vgps8":
                            nc.gpsimd.dma_start(
                                mt[:],
                                b_dram[i * P:(i + 1) * P, 0:QW]
                                .bitcast(mybir.dt.float32))
                        else:
                            nc.sync.dma_start(
                                mt[:],
                                b_dram[i * P:(i + 1) * P, 0:QW]
                                .bitcast(mybir.dt.float32))
            bt_fixed = None
            ot_fixed = None

            def _asrc_f32(j):
                src = a_sb[:, 0, 4 * j:4 * j + 4, :]
                if dt_in == mybir.dt.float32r:
                    src = src.bitcast(mybir.dt.float32)
                return src

            if variant == "vstore":
                ot_fixed = opool.tile([P, QW], mybir.dt.float32,
                                      name="ot_fixed")
                nc.vector.tensor_copy(
                    ot_fixed[:].rearrange("p (a b) -> p a b", a=4),
                    _asrc_f32(0))

            def _bsrc(h, kg, q):
                return b_dram[
                    (h * NQ + q) * P:(h * NQ + q + 1) * P,
                    4 * kg * QW:(4 * kg + 4) * QW,
                ].rearrange("p (ko n) -> p ko n", ko=4)

            def _load_diag_chunk(bt, q):
                # per k-row load only the valid columns [128i, 512) -
                # below-diagonal 128-blocks of B are zero
                for h in range(two):
                    for i in range(4):
                        row = (h * NQ + q) * P
                        col = (4 * q + i) * QW + 128 * i
                        nc.sync.dma_start(
                            bt[:, h, i, 128 * i:],
                            b_dram[row:row + P, col:col + QW - 128 * i])

            # q=0's only chunk (0.6MB) is consumed last (Q_ORDER ends on 0):
            # prefetch it into a dedicated buffer at the start so the tail
            # never waits on DMA
            # (tried: prefetching q=0's chunk at the head — model-worse by
            # 1.8us, the DMA stream is saturated so early bytes displace
            # the critical sequence)
            bt_q0 = None

            # q order: q=1 first (ready after the small A-head load), then
            # heaviest-to-lightest so the schedule drains into the tiny q=0
            # tail (4 matmuls + 1 copy + 1 store). Model-swept optimum.
            Q_ORDER = globals().get("_Q_ORDER_OVERRIDE") or \
                [1, 7, 6, 5, 4, 3, 2, 0]
            for _r, q in ([] if micro else
                          [(r, q) for r in range(rep) for q in Q_ORDER]):
                act = [j for j in range(NSLOT) if 4 * q + 4 > 8 * j]
                psums = {
                    j: pspool.tile([P, QW], mybir.dt.float32, tag="ps",
                                   name=f"ps_{_r}_{q}_{j}")
                    for j in act
                } if do_mm else {}
                kend = 4 * q + 3
                for kg in range(q + 1):
                    if do_mm and not do_bdma:
                        if bt_fixed is None:
                            bt_fixed = bpool.tile([P, two, 4, QW], dt_in,
                                                  tag="bt", name="bt_fixed")
                            for h in range(two):
                                nc.sync.dma_start(bt_fixed[:, h],
                                                  _bsrc(h, 0, 0))
                        bt = bt_fixed
                    elif do_bdma or variant == "vmin":
                        if variant == "vmin" and kg > 0:
                            continue
                        if bt_q0 is not None and q == 0:
                            bt = bt_q0
                        else:
                            bt = bpool.tile([P, two, 4, QW], dt_in,
                                            tag="bt")
                            if kg == q:
                                _load_diag_chunk(bt, q)
                            else:
                                for h in range(two):
                                    nc.sync.dma_start(bt[:, h],
                                                      _bsrc(h, kg, q))
                    else:
                        continue
                    if not do_mm:
                        continue
                    for i in range(4):
                        k = 4 * kg + i
                        # on the diagonal chunk only columns >= 128i are
                        # valid in SBUF (and B is zero left of them anyway)
                        c0 = 128 * i if kg == q else 0
                        for j in act:
                            if k < KSTART[j]:
                                continue
                            idx = AOFF[j] + (k - KSTART[j])
                            first = k == KSTART[j]
                            last = k == kend
                            if two == 1:
                                nc.tensor.matmul(
                                    psums[j][:, c0:], a_sb[:, 0, idx, :],
                                    bt[:, 0, i, c0:],
                                    start=first, stop=last)
                            else:
                                # hi@hi, hi@lo, lo@hi
                                for n3, (ha, hb) in enumerate(
                                        [(0, 0), (0, 1), (1, 0)]):
                                    nc.tensor.matmul(
                                        psums[j][:, c0:],
                                        a_sb[:, ha, idx, :],
                                        bt[:, hb, i, c0:],
                                        start=first and n3 == 0,
                                        stop=last and n3 == 2)
                for j in act:
                    if not (do_copy or do_store):
                        continue
                    t = PAIRS.index((j, q))
                    if variant == "vstore":
                        nc.sync.dma_start(
                            c_dram[t * P:(t + 1) * P, :], ot_fixed[:])
                        continue
                    ot = opool.tile([P, QW], mybir.dt.float32, tag="ot")
                    if do_mm:
                        nc.vector.tensor_copy(ot[:], psums[j][:])
                    else:
                        nc.vector.tensor_copy(
                            ot[:].rearrange("p (a b) -> p a b", a=4),
                            _asrc_f32(j))
                    if do_store:
                        # scalar (ACT) HWDGE ring: keeps compute-gated output
                        # stores out of the B-stream's SP FIFO
                        nc.scalar.dma_start(
                            c_dram[t * P:(t + 1) * P, :], ot[:])
    nc.compile()
    _nc_cache[(mode, rep, variant)] = nc
    return nc


def _split_bf16(x):
    import ml_dtypes
    hi = x.astype(ml_dtypes.bfloat16)
    lo = (x - hi.astype(np.float32)).astype(ml_dtypes.bfloat16)
    return hi, lo


def pack_inputs(A, B, mode=MODE):
    """Build per-core in_maps (partition-major packed layouts)."""
    A = np.ascontiguousarray(np.asarray(A, dtype=np.float32))
    B = np.ascontiguousarray(np.asarray(B, dtype=np.float32))
    two = 2 if mode == "bf16x3" else 1

    # B[128k+p, 512q+n] -> Bp[q, p, k, n] -> [NQ*P, NKT*QW]
    def _pack_b(x):
        return np.ascontiguousarray(
            x.reshape(NKT, P, NQ, QW).transpose(2, 1, 0, 3)
        ).reshape(NQ * P, NKT * QW)

    if mode == "bf16x3":
        hi, lo = _split_bf16(B)
        b_all = np.concatenate([_pack_b(hi), _pack_b(lo)], axis=0)
    else:
        b_all = _pack_b(B)

    in_maps = []
    for c in range(NCORES):
        ap = np.zeros((ATOT, P, P), np.float32)
        for j in range(NSLOT):
            b = 8 * j + c
            rb = P * b
            for k in range(max(KSTART[j], b), NKT):
                ap[AOFF[j] + k - KSTART[j]] = \
                    A[rb:rb + P, P * k:P * k + P].T
        # [t, p, m] -> [p, t, m] -> [P, ATOT*P]
        def _pack_a(x):
            return np.ascontiguousarray(
                x.transpose(1, 0, 2)).reshape(P, ATOT * P)

        if mode == "bf16x3":
            hi, lo = _split_bf16(ap)
            apk = np.concatenate([_pack_a(hi), _pack_a(lo)], axis=0)
        else:
            apk = _pack_a(ap)
        in_maps.append({"Apack": apk, "B": b_all})
    return in_maps


def unpack_output(results):
    C = np.zeros((N, N), np.float32)
    for c, r in enumerate(results):
        co = np.asarray(r["Cout"]).reshape(NT, P, QW)
        for t, (j, q) in enumerate(PAIRS):
            b = 8 * j + c
            C[P * b:P * b + P, QW * q:QW * q + QW] = co[t]
    return C


def kernel(A, B):
    nc = build_nc(MODE)
    in_maps = pack_inputs(A, B, MODE)
    res = bass_utils.run_bass_kernel_spmd(
        nc, in_maps, core_ids=list(range(NCORES)), trace=False)
    return unpack_output(res.results)



# revision 33
# speedup vs baseline: 2.0383x; 2.0383x over previous
"""Trainium2 Bass kernel: C = triu(A @ B), A/B upper-triangular 4096x4096 fp32.

Strategy (2D: 4 row-groups x 2 col-groups, SPMD single program, bf16):
  * Core c -> (r = c%4, s = c//4). Core owns row blocks {4j + r, j=0..7}
    (8 slots of 128 rows) and output column tiles q in {2m+s, m=0..3}
    (4 slots of 512 cols).  vs. the 1D row-parallel layout this halves
    the per-core B stream (only its own q columns) at the cost of 2x A.
  * One uniform schedule for all cores: for m-slot and chunk kg (4
    k-tiles of 128), accumulate k in [4j, 8m+7] into psum[j] for every
    active slot j <= 2m+1.  Per-core variation lives entirely in the
    DATA: A^T tiles with k < own-block are zero-filled by the host, and
    B's below-diagonal blocks are zero in the source matrix, so padded
    matmuls contribute exact zeros.  Since A and B are upper-triangular
    the lower triangle of C comes out exactly 0 - no masking needed.
  * bf16 inputs (rel-err ~2e-3 vs 2e-2 budget) halve HBM bytes; C is
    stored bf16 too (rel-err ~4e-3).  Per-core HBM traffic ~16.6 MB,
    PE ~128k columns: balanced at ~53us each.
  * The last chunk of each m (kg = 2m+1) is the diagonal chunk for s=1
    cores and all-zero for s=0 cores: loaded with the triangle pattern
    (cols >= 128i of k-row i) and matmul'd width-masked - correct for
    both.  A^T is packed in first-use order and streamed in 4 stages.
"""

import numpy as np
from contextlib import ExitStack

import concourse.mybir as mybir
import concourse.tile as tile
from concourse import bacc, bass_utils

N = 4096
P = 128
NCORES = 8
GR = 4             # row groups (cores per column group)
GS = 2             # column groups
NJ = 8             # row-block slots per core (32 blocks / GR)
NM = 4             # 512-wide output column slots per core (8 q-tiles / GS)
QW = 512
NKT = 32           # 128-wide k tiles

# (slot, m) pairs the program computes/writes, in emission order
PAIRS = [(j, m) for m in range(NM) for j in range(2 * m + 2)]
NT = len(PAIRS)    # 20 output tiles of 128x512 per core


def _kgs(m):
    """Chunk order within an m-slot.  Descending puts the k-tiles shared
    by many slots first (more PE work per DMA byte early) and completes
    psum j exactly at chunk kg=j, spreading the copy/store drain across
    the whole m instead of bunching it at the end."""
    r = range(2 * m + 2)
    return list(reversed(r)) if KG_DESC else list(r)


def _a_layout(order):
    """A^T tiles in exact first-use order of the emission schedule; group
    g holds the tiles first needed by chunk g, so A streams just-in-time
    interleaved with the B chunks."""
    idx = {}
    groups = []
    t = 0
    for m in order:
        for kg in _kgs(m):
            t0 = t
            for i in range(4):
                k = 4 * kg + i
                for j in range(2 * m + 2):
                    if k >= 4 * j and (j, k) not in idx:
                        idx[(j, k)] = t
                        t += 1
            groups.append((t0, t))
    return idx, groups


def set_order(order, kg_desc=None):
    """Set the m emission order and recompute the derived layout."""
    global M_ORDER, ATIDX, AGROUPS, ATOT, DRAIN_MS, KG_DESC
    if kg_desc is not None:
        KG_DESC = kg_desc
    M_ORDER = list(order)
    ATIDX, AGROUPS = _a_layout(M_ORDER)
    ATOT = max(t1 for _, t1 in AGROUPS)    # 144 packed A tiles per core
    DRAIN_MS = set(M_ORDER[-2:])
    _nc_cache.clear()


_nc_cache = {}
KG_DESC = True
# m emission order: big-PE m=3 late so DMA prefetch runs ahead, tiny
# m=0 (2 output tiles) last so the store-drain tail is short
set_order([1, 2, 3, 0])

# matmul dtype mode: "bf16" (single pass, ~8-bit mantissa, half the HBM
# bytes), "fp32r" (~11-bit mantissa, 4x PE cost at width<256), "fp32"
# (exact, 4x slower PE)
MODE = "bf16"
C_BF16 = True      # store C as bf16 (halves output traffic)

# pool buffer counts (double/triple buffering)
BUFS_B = 6
BUFS_O = 6
BUFS_PS = 8

# drain engine assignment, cycled per tile: copy 0=DVE 1=ACT(scalar.copy),
# store 0=ACT ring 1=SP ring
DRAIN_COPY = [0]
DRAIN_STORE = [0, 1]


def build_nc(mode=MODE, rep=1, variant="full"):
    """rep>1 repeats the whole compute (for dispatch-overhead-cancelling
    timing).  variant: "full" | "nomm" (DMAs only) | "nodma" (matmuls
    only)."""
    if (mode, rep, variant) in _nc_cache:
        return _nc_cache[(mode, rep, variant)]
    dt_in = {
        "bf16": mybir.dt.bfloat16,
        "fp32r": mybir.dt.float32r,
        "fp32": mybir.dt.float32,
    }[mode]
    dt_c = mybir.dt.bfloat16 if C_BF16 else mybir.dt.float32

    nc = bacc.Bacc("TRN2", target_bir_lowering=False, debug=False,
                   num_devices=NCORES)
    # partition-major packed layouts (see pack_inputs): per-partition data
    # is contiguous so every DMA is 128 descriptors of large runs.
    # Apack row = p(k-within-tile), col = t*P + m  (first-use tile order)
    a_dram = nc.dram_tensor("Apack", [P, ATOT * P], dt_in,
                            kind="ExternalInput").ap()
    # B row = m*P + p, col = k*QW + n   (core's q column = 2m + s)
    b_dram = nc.dram_tensor("B", [NM * P, NKT * QW], dt_in,
                            kind="ExternalInput").ap()
    c_dram = nc.dram_tensor("Cout", [NT * P, QW], dt_c,
                            kind="ExternalOutput").ap()

    with tile.TileContext(nc) as tc:
        with ExitStack() as ctx:
            apool = ctx.enter_context(tc.tile_pool(name="apool", bufs=1))
            bpool = ctx.enter_context(tc.tile_pool(name="bpool", bufs=BUFS_B))
            opool = ctx.enter_context(tc.tile_pool(name="opool", bufs=BUFS_O))
            pspool = ctx.enter_context(
                tc.tile_pool(name="pspool", bufs=BUFS_PS, space="PSUM"))

            do_bdma = variant in ("full", "nomm")
            do_mm = variant in ("full", "nodma")
            do_copy = variant in ("full", "nomm", "nodma")
            do_store = variant in ("full", "nomm", "nodma")

            a_sb = apool.tile([P, ATOT, P], dt_in)

            def _load_a_group(g):
                t0, t1 = AGROUPS[g]
                if t0 == t1:
                    return
                nc.sync.dma_start(
                    a_sb[:, t0:t1, :],
                    a_dram[:, t0 * P:t1 * P].rearrange(
                        "p (t m) -> p t m", m=P))

            def _load_chunk(bt, m, kg):
                if kg == 2 * m + 1:
                    # last chunk: diagonal for s=1, all-zero for s=0 -
                    # per k-row load only cols >= 128i (the rest of the
                    # tile is stale and masked out of the matmuls)
                    for i in range(4):
                        col = (4 * kg + i) * QW + 128 * i
                        nc.sync.dma_start(
                            bt[:, i, 128 * i:],
                            b_dram[m * P:(m + 1) * P,
                                   col:col + QW - 128 * i])
                else:
                    nc.sync.dma_start(
                        bt[:],
                        b_dram[m * P:(m + 1) * P,
                               4 * kg * QW:(4 * kg + 4) * QW].rearrange(
                                   "p (ko n) -> p ko n", ko=4))

            bt_fixed = None
            for _r in range(rep):
                g = -1
                for m in M_ORDER:
                    act = list(range(2 * m + 2))
                    psums = {
                        j: pspool.tile([P, QW], mybir.dt.float32, tag="ps",
                                       name=f"ps_{_r}_{m}_{j}")
                        for j in act
                    } if do_mm else {}
                    kend = 8 * m + 7
                    for kg in range(2 * m + 2):
                        g += 1
                        if _r == 0:
                            _load_a_group(g)
                        if do_bdma:
                            bt = bpool.tile([P, 4, QW], dt_in, tag="bt")
                            _load_chunk(bt, m, kg)
                        elif do_mm:
                            if bt_fixed is None:
                                bt_fixed = bpool.tile([P, 4, QW], dt_in,
                                                      tag="bt",
                                                      name="bt_fixed")
                                _load_chunk(bt_fixed, 0, 0)
                            bt = bt_fixed
                        else:
                            continue
                        if not do_mm:
                            continue
                        last_chunk = kg == 2 * m + 1
                        if last_chunk:
                            # j-major so each psum's stop matmul lands
                            # early and the copy/store drain pipelines
                            for j in act:
                                for i in range(4):
                                    k = 4 * kg + i
                                    if k < 4 * j:
                                        continue
                                    c0 = 128 * i
                                    nc.tensor.matmul(
                                        psums[j][:, c0:],
                                        a_sb[:, ATIDX[(j, k)], :],
                                        bt[:, i, c0:],
                                        start=k == 4 * j, stop=k == kend)
                        else:
                            for i in range(4):
                                k = 4 * kg + i
                                for j in act:
                                    if k < 4 * j:
                                        continue
                                    nc.tensor.matmul(
                                        psums[j][:], a_sb[:, ATIDX[(j, k)], :],
                                        bt[:, i, :],
                                        start=k == 4 * j, stop=k == kend)
                    drain = m in DRAIN_MS
                    for nj, j in enumerate(act):
                        if not (do_copy or do_store):
                            continue
                        t = PAIRS.index((j, m))
                        ot = opool.tile([P, QW], dt_c, tag="ot")
                        if do_mm:
                            # in the drain, spread copies over DVE+ACT so
                            # the tail isn't a serial DVE chain (GPSIMD
                            # cannot read PSUM)
                            if drain and DRAIN_COPY[nj % len(DRAIN_COPY)]:
                                nc.scalar.copy(ot[:], psums[j][:])
                            else:
                                nc.vector.tensor_copy(ot[:], psums[j][:])
                        else:
                            src = a_sb[:, 4 * j:4 * j + 4, :]
                            if dt_in == mybir.dt.float32r:
                                src = src.bitcast(mybir.dt.float32)
                            nc.vector.tensor_copy(
                                ot[:].rearrange("p (a b) -> p a b", a=4),
                                src)
                        if do_store:
                            # ACT HWDGE ring keeps compute-gated stores out
                            # of the B-stream's SP FIFO; in the drain the
                            # B stream is done, so SP is free too
                            ring = (nc.sync if drain
                                    and DRAIN_STORE[nj % len(DRAIN_STORE)]
                                    else nc.scalar)
                            ring.dma_start(
                                c_dram[t * P:(t + 1) * P, :], ot[:])
    nc.compile()
    _nc_cache[(mode, rep, variant)] = nc
    return nc


def pack_inputs(A, B, mode=MODE):
    """Build per-core in_maps (partition-major packed layouts)."""
    import ml_dtypes
    A = np.ascontiguousarray(np.asarray(A, dtype=np.float32))
    B = np.ascontiguousarray(np.asarray(B, dtype=np.float32))
    dt_np = {
        "bf16": ml_dtypes.bfloat16,
        "fp32r": np.float32,
        "fp32": np.float32,
    }[mode]

    # B[128k+p, 512q+n] -> per col-group s: Bp[m, p, k, n] with q = 2m+s
    b_packs = []
    b4 = B.reshape(NKT, P, NQ_G := 8, QW)
    for s in range(GS):
        qsel = [2 * m + s for m in range(NM)]
        b_packs.append(np.ascontiguousarray(
            b4[:, :, qsel, :].transpose(2, 1, 0, 3).astype(dt_np)
        ).reshape(NM * P, NKT * QW))

    in_maps = []
    for c in range(NCORES):
        r, s = c % GR, c // GR
        ap = np.zeros((ATOT, P, P), np.float32)
        for (j, k), t in ATIDX.items():
            b = GR * j + r
            if k >= b:
                ap[t] = A[P * b:P * b + P, P * k:P * k + P].T
        # [t, p, m] -> [p, t, m] -> [P, ATOT*P]
        apk = np.ascontiguousarray(
            ap.astype(dt_np).transpose(1, 0, 2)).reshape(P, ATOT * P)
        in_maps.append({"Apack": apk, "B": b_packs[s]})
    return in_maps


def unpack_output(results):
    C = np.zeros((N, N), np.float32)
    for c, r in enumerate(results):
        rr, s = c % GR, c // GR
        co = np.asarray(r["Cout"]).astype(np.float32).reshape(NT, P, QW)
        for t, (j, m) in enumerate(PAIRS):
            b = GR * j + rr
            q = 2 * m + s
            if P * b >= QW * (q + 1):
                continue               # fully below-diagonal tile: zeros
            C[P * b:P * b + P, QW * q:QW * q + QW] = co[t]
    return C


def kernel(A, B):
    nc = build_nc(MODE)
    in_maps = pack_inputs(A, B, MODE)
    res = bass_utils.run_bass_kernel_spmd(
        nc, in_maps, core_ids=list(range(NCORES)), trace=False)
    return unpack_output(res.results)


# revision 43
# speedup vs baseline: 2.0507x; 1.0061x over previous
"""Trainium2 Bass kernel: C = triu(A @ B), A/B upper-triangular 4096x4096 fp32.

Strategy (2D: 4 row-groups x 2 col-groups, SPMD single program, bf16):
  * Core c -> (r = c%4, s = c//4). Core owns row blocks {4j + r, j=0..7}
    (8 slots of 128 rows) and output column tiles q in {2m+s, m=0..3}
    (4 slots of 512 cols).  vs. the 1D row-parallel layout this halves
    the per-core B stream (only its own q columns) at the cost of 2x A.
  * One uniform schedule for all cores: for m-slot and chunk kg (4
    k-tiles of 128), accumulate k in [4j, 8m+7] into psum[j] for every
    active slot j <= 2m+1.  Per-core variation lives entirely in the
    DATA: A^T tiles with k < own-block are zero-filled by the host, and
    B's below-diagonal blocks are zero in the source matrix, so padded
    matmuls contribute exact zeros.  Since A and B are upper-triangular
    the lower triangle of C comes out exactly 0 - no masking needed.
  * bf16 inputs (rel-err ~2e-3 vs 2e-2 budget) halve HBM bytes; C is
    stored bf16 too (rel-err ~4e-3).  Per-core HBM traffic ~16.6 MB,
    PE ~128k columns: balanced at ~53us each.
  * The last chunk of each m (kg = 2m+1) is the diagonal chunk for s=1
    cores and all-zero for s=0 cores: loaded with the triangle pattern
    (cols >= 128i of k-row i) and matmul'd width-masked - correct for
    both.  A^T is packed in first-use order and streamed in 4 stages.
"""

import numpy as np
from contextlib import ExitStack

import concourse.mybir as mybir
import concourse.tile as tile
from concourse import bacc, bass_utils

N = 4096
P = 128
NCORES = 8
GR = 4             # row groups (cores per column group)
GS = 2             # column groups
NJ = 8             # row-block slots per core (32 blocks / GR)
NM = 4             # 512-wide output column slots per core (8 q-tiles / GS)
QW = 512
NKT = 32           # 128-wide k tiles

# (slot, m) pairs the program computes/writes, in emission order
PAIRS = [(j, m) for m in range(NM) for j in range(2 * m + 2)]
NT = len(PAIRS)    # 20 output tiles of 128x512 per core


def _kgs(m):
    """Chunk order within an m-slot.  Descending puts the k-tiles shared
    by many slots first (more PE work per DMA byte early) and completes
    psum j exactly at chunk kg=j, spreading the copy/store drain across
    the whole m instead of bunching it at the end."""
    r = range(2 * m + 2)
    return list(reversed(r)) if m in KG_DESC else list(r)


def _a_layout(order):
    """A^T tiles in exact first-use order of the emission schedule; group
    g holds the tiles first needed by chunk g, so A streams just-in-time
    interleaved with the B chunks."""
    idx = {}
    groups = []
    t = 0
    for m in order:
        for kg in _kgs(m):
            t0 = t
            for i in range(4):
                k = 4 * kg + i
                for j in range(2 * m + 2):
                    if k >= 4 * j and (j, k) not in idx:
                        idx[(j, k)] = t
                        t += 1
            groups.append((t0, t))
    return idx, groups


def set_order(order, kg_desc=None):
    """Set the m emission order and recompute the derived layout."""
    global M_ORDER, ATIDX, AGROUPS, ATOT, DRAIN_MS, KG_DESC
    if kg_desc is not None:
        KG_DESC = set(kg_desc)
    M_ORDER = list(order)
    ATIDX, AGROUPS = _a_layout(M_ORDER)
    ATOT = max(t1 for _, t1 in AGROUPS)    # 144 packed A tiles per core
    DRAIN_MS = set(M_ORDER[-2:])
    _nc_cache.clear()


_nc_cache = {}
KG_DESC = set()
# m emission order: big-PE m=3 late so DMA prefetch runs ahead, tiny
# m=0 (2 output tiles) last so the store-drain tail is short; kg
# descending inside m=2 and m=0 (model-swept optimum)
set_order([1, 3, 2, 0], kg_desc={0, 2})

# matmul dtype mode: "bf16" (single pass, ~8-bit mantissa, half the HBM
# bytes), "fp32r" (~11-bit mantissa, 4x PE cost at width<256), "fp32"
# (exact, 4x slower PE)
MODE = "bf16"
C_BF16 = True      # store C as bf16 (halves output traffic)

# pool buffer counts (double/triple buffering)
BUFS_B = 6
BUFS_O = 6
BUFS_PS = 8

# drain engine assignment, cycled per tile: copy 0=DVE 1=ACT(scalar.copy),
# store 0=ACT ring 1=SP ring
DRAIN_COPY = [0]
DRAIN_STORE = [0, 1]


def build_nc(mode=MODE, rep=1, variant="full"):
    """rep>1 repeats the whole compute (for dispatch-overhead-cancelling
    timing).  variant: "full" | "nomm" (DMAs only) | "nodma" (matmuls
    only)."""
    if (mode, rep, variant) in _nc_cache:
        return _nc_cache[(mode, rep, variant)]
    dt_in = {
        "bf16": mybir.dt.bfloat16,
        "fp32r": mybir.dt.float32r,
        "fp32": mybir.dt.float32,
    }[mode]
    dt_c = mybir.dt.bfloat16 if C_BF16 else mybir.dt.float32

    nc = bacc.Bacc("TRN2", target_bir_lowering=False, debug=False,
                   num_devices=NCORES)
    # partition-major packed layouts (see pack_inputs): per-partition data
    # is contiguous so every DMA is 128 descriptors of large runs.
    # Apack row = p(k-within-tile), col = t*P + m  (first-use tile order)
    a_dram = nc.dram_tensor("Apack", [P, ATOT * P], dt_in,
                            kind="ExternalInput").ap()
    # B row = m*P + p, col = k*QW + n   (core's q column = 2m + s)
    b_dram = nc.dram_tensor("B", [NM * P, NKT * QW], dt_in,
                            kind="ExternalInput").ap()
    c_dram = nc.dram_tensor("Cout", [NT * P, QW], dt_c,
                            kind="ExternalOutput").ap()

    with tile.TileContext(nc) as tc:
        with ExitStack() as ctx:
            apool = ctx.enter_context(tc.tile_pool(name="apool", bufs=1))
            bpool = ctx.enter_context(tc.tile_pool(name="bpool", bufs=BUFS_B))
            opool = ctx.enter_context(tc.tile_pool(name="opool", bufs=BUFS_O))
            pspool = ctx.enter_context(
                tc.tile_pool(name="pspool", bufs=BUFS_PS, space="PSUM"))

            do_bdma = variant in ("full", "nomm")
            do_mm = variant in ("full", "nodma")
            do_copy = variant in ("full", "nomm", "nodma")
            do_store = variant in ("full", "nomm", "nodma")

            a_sb = apool.tile([P, ATOT, P], dt_in)

            def _load_a_group(g):
                t0, t1 = AGROUPS[g]
                if t0 == t1:
                    return
                nc.sync.dma_start(
                    a_sb[:, t0:t1, :],
                    a_dram[:, t0 * P:t1 * P].rearrange(
                        "p (t m) -> p t m", m=P))

            def _load_chunk(bt, m, kg, ring=None):
                ring = ring or nc.sync
                if kg == 2 * m + 1:
                    # last chunk: diagonal for s=1, all-zero for s=0 -
                    # per k-row load only cols >= 128i (the rest of the
                    # tile is stale and masked out of the matmuls)
                    for i in range(4):
                        col = (4 * kg + i) * QW + 128 * i
                        ring.dma_start(
                            bt[:, i, 128 * i:],
                            b_dram[m * P:(m + 1) * P,
                                   col:col + QW - 128 * i])
                else:
                    ring.dma_start(
                        bt[:],
                        b_dram[m * P:(m + 1) * P,
                               4 * kg * QW:(4 * kg + 4) * QW].rearrange(
                                   "p (ko n) -> p ko n", ko=4))

            def _emit_out(m, j, nj, psums):
                # copy psum j to SBUF (dtype convert) and store the tile
                drain = m in DRAIN_MS
                t = PAIRS.index((j, m))
                ot = opool.tile([P, QW], dt_c, tag="ot")
                if do_mm:
                    # in the drain, optionally spread copies over DVE+ACT
                    # so the tail isn't a serial DVE chain (GPSIMD cannot
                    # read PSUM)
                    if drain and DRAIN_COPY[nj % len(DRAIN_COPY)]:
                        nc.scalar.copy(ot[:], psums[j][:])
                    else:
                        nc.vector.tensor_copy(ot[:], psums[j][:])
                else:
                    src = a_sb[:, 4 * j:4 * j + 4, :]
                    if dt_in == mybir.dt.float32r:
                        src = src.bitcast(mybir.dt.float32)
                    nc.vector.tensor_copy(
                        ot[:].rearrange("p (a b) -> p a b", a=4), src)
                if do_store:
                    # ACT HWDGE ring keeps compute-gated stores out of the
                    # B-stream's SP FIFO; in the drain the B stream is
                    # done, so SP is free too
                    ring = (nc.sync if drain
                            and DRAIN_STORE[nj % len(DRAIN_STORE)]
                            else nc.scalar)
                    ring.dma_start(c_dram[t * P:(t + 1) * P, :], ot[:])

            bt_fixed = None
            for _r in range(rep):
                g = -1
                for m in M_ORDER:
                    act = list(range(2 * m + 2))
                    psums = {
                        j: pspool.tile([P, QW], mybir.dt.float32, tag="ps",
                                       name=f"ps_{_r}_{m}_{j}")
                        for j in act
                    } if do_mm else {}
                    done = []
                    for kg in _kgs(m):
                        g += 1
                        if _r == 0:
                            _load_a_group(g)
                        if do_bdma:
                            bt = bpool.tile([P, 4, QW], dt_in, tag="bt")
                            _load_chunk(bt, m, kg)
                        elif do_mm:
                            if bt_fixed is None:
                                bt_fixed = bpool.tile([P, 4, QW], dt_in,
                                                      tag="bt",
                                                      name="bt_fixed")
                                _load_chunk(bt_fixed, 0, 0)
                            bt = bt_fixed
                        if do_mm and (do_bdma or bt_fixed is not None):
                            masked = kg == 2 * m + 1
                            # j-major within the masked chunk so psum
                            # stops/starts stagger
                            for j in (act if masked else [None]):
                                for i in range(4):
                                    k = 4 * kg + i
                                    for jj in ([j] if masked else act):
                                        if k < 4 * jj:
                                            continue
                                        c0 = 128 * i if masked else 0
                                        desc = m in KG_DESC
                                        first = (k == 8 * m + 4 if desc
                                                 else k == 4 * jj)
                                        last = (k == 4 * jj + 3 if desc
                                                else k == 8 * m + 7)
                                        nc.tensor.matmul(
                                            psums[jj][:, c0:],
                                            a_sb[:, ATIDX[(jj, k)], :],
                                            bt[:, i, c0:],
                                            start=first, stop=last)
                        if m in KG_DESC and do_mm and \
                                (do_copy or do_store):
                            # chunk kg completes exactly psum j = kg
                            _emit_out(m, kg, len(done), psums)
                            done.append(kg)
                    if not (do_copy or do_store):
                        continue
                    for nj, j in enumerate(act):
                        if j in done:
                            continue
                        if do_mm and not (do_bdma or bt_fixed is not None):
                            continue
                        _emit_out(m, j, nj, psums)
    nc.compile()
    _nc_cache[(mode, rep, variant)] = nc
    return nc


def pack_inputs(A, B, mode=MODE):
    """Build per-core in_maps (partition-major packed layouts)."""
    import ml_dtypes
    A = np.ascontiguousarray(np.asarray(A, dtype=np.float32))
    B = np.ascontiguousarray(np.asarray(B, dtype=np.float32))
    dt_np = {
        "bf16": ml_dtypes.bfloat16,
        "fp32r": np.float32,
        "fp32": np.float32,
    }[mode]

    # B[128k+p, 512q+n] -> per col-group s: Bp[m, p, k, n] with q = 2m+s
    b_packs = []
    b4 = B.reshape(NKT, P, NQ_G := 8, QW)
    for s in range(GS):
        qsel = [2 * m + s for m in range(NM)]
        b_packs.append(np.ascontiguousarray(
            b4[:, :, qsel, :].transpose(2, 1, 0, 3).astype(dt_np)
        ).reshape(NM * P, NKT * QW))

    in_maps = []
    for c in range(NCORES):
        r, s = c % GR, c // GR
        ap = np.zeros((ATOT, P, P), np.float32)
        for (j, k), t in ATIDX.items():
            b = GR * j + r
            if k >= b:
                ap[t] = A[P * b:P * b + P, P * k:P * k + P].T
        # [t, p, m] -> [p, t, m] -> [P, ATOT*P]
        apk = np.ascontiguousarray(
            ap.astype(dt_np).transpose(1, 0, 2)).reshape(P, ATOT * P)
        in_maps.append({"Apack": apk, "B": b_packs[s]})
    return in_maps


def unpack_output(results):
    C = np.zeros((N, N), np.float32)
    for c, r in enumerate(results):
        rr, s = c % GR, c // GR
        co = np.asarray(r["Cout"]).astype(np.float32).reshape(NT, P, QW)
        for t, (j, m) in enumerate(PAIRS):
            b = GR * j + rr
            q = 2 * m + s
            if P * b >= QW * (q + 1):
                continue               # fully below-diagonal tile: zeros
            C[P * b:P * b + P, QW * q:QW * q + QW] = co[t]
    return C


def kernel(A, B):
    nc = build_nc(MODE)
    in_maps = pack_inputs(A, B, MODE)
    res = bass_utils.run_bass_kernel_spmd(
        nc, in_maps, core_ids=list(range(NCORES)), trace=False)
    return unpack_output(res.results)


# revision 51
# speedup vs baseline: 2.0555x; 1.0023x over previous
"""Trainium2 Bass kernel: C = triu(A @ B), A/B upper-triangular 4096x4096 fp32.

Strategy (2D: 4 row-groups x 2 col-groups, SPMD single program, bf16):
  * Core c -> (r = c%4, s = c//4). Core owns row blocks {4j + r, j=0..7}
    (8 slots of 128 rows) and output column tiles q in {2m+s, m=0..3}
    (4 slots of 512 cols).  vs. the 1D row-parallel layout this halves
    the per-core B stream (only its own q columns) at the cost of 2x A.
  * One uniform schedule for all cores: for m-slot and chunk kg (4
    k-tiles of 128), accumulate k in [4j, 8m+7] into psum[j] for every
    active slot j <= 2m+1.  Per-core variation lives entirely in the
    DATA: A^T tiles with k < own-block are zero-filled by the host, and
    B's below-diagonal blocks are zero in the source matrix, so padded
    matmuls contribute exact zeros.  Since A and B are upper-triangular
    the lower triangle of C comes out exactly 0 - no masking needed.
  * bf16 inputs (rel-err ~2e-3 vs 2e-2 budget) halve HBM bytes; C is
    stored bf16 too (rel-err ~4e-3).  Per-core HBM traffic ~16.6 MB,
    PE ~128k columns: balanced at ~53us each.
  * The masked chunk of each m (kg = 2m+1) is the diagonal chunk for
    s=1 cores and all-zero for s=0 cores: loaded with the triangle
    pattern (cols >= 128i of k-row i) and matmul'd width-masked -
    correct for both.  A^T is packed in exact first-use order of the
    emission schedule and streamed just-in-time, one group per chunk.
  * Schedule (model-swept): m order [1,3,2,0] - tiny m=0 last so the
    copy/store drain tail is short; kg descending inside m=2 and m=0,
    which completes psum j exactly at chunk kg=j and spreads the drain.
"""

import numpy as np
from contextlib import ExitStack

import concourse.mybir as mybir
import concourse.tile as tile
from concourse import bacc, bass_utils

N = 4096
P = 128
NCORES = 8
GR = 4             # row groups (cores per column group)
GS = 2             # column groups
NJ = 8             # row-block slots per core (32 blocks / GR)
NM = 4             # 512-wide output column slots per core (8 q-tiles / GS)
QW = 512
NKT = 32           # 128-wide k tiles

# (slot, m) pairs the program computes/writes, in emission order
PAIRS = [(j, m) for m in range(NM) for j in range(2 * m + 2)]
NT = len(PAIRS)    # 20 output tiles of 128x512 per core


def _kgs(m):
    """Chunk order within an m-slot.  Descending puts the k-tiles shared
    by many slots first (more PE work per DMA byte early) and completes
    psum j exactly at chunk kg=j, spreading the copy/store drain across
    the whole m instead of bunching it at the end."""
    r = range(2 * m + 2)
    return list(reversed(r)) if m in KG_DESC else list(r)


def _a_layout(seq):
    """A^T tiles in exact first-use order of the emission schedule; group
    g holds the tiles first needed by chunk g, so A streams just-in-time
    interleaved with the B chunks."""
    idx = {}
    groups = []
    t = 0
    for m, kg in seq:
        t0 = t
        for i in range(4):
            k = 4 * kg + i
            for j in range(2 * m + 2):
                if k >= 4 * j and (j, k) not in idx:
                    idx[(j, k)] = t
                    t += 1
        groups.append((t0, t))
    return idx, groups


def set_order(order, kg_desc=None, seq=None):
    """Set the m emission order and recompute the derived layout.  seq
    overrides the chunk sequence (must preserve within-m kg order of
    _kgs and contain every (m, kg) exactly once)."""
    global M_ORDER, ATIDX, AGROUPS, ATOT, DRAIN_MS, KG_DESC, CHUNK_SEQ
    if kg_desc is not None:
        KG_DESC = set(kg_desc)
    M_ORDER = list(order)
    CHUNK_SEQ = (list(seq) if seq is not None else
                 [(m, kg) for m in M_ORDER for kg in _kgs(m)])
    ATIDX, AGROUPS = _a_layout(CHUNK_SEQ)
    ATOT = max(t1 for _, t1 in AGROUPS)    # 144 packed A tiles per core
    DRAIN_MS = set(M_ORDER[-2:])
    _nc_cache.clear()


_nc_cache = {}
KG_DESC = set()
# m emission order: big-PE m=3 late so DMA prefetch runs ahead, tiny
# m=0 (2 output tiles) last so the store-drain tail is short; kg
# descending inside m=2 and m=0 (model-swept optimum)
set_order([1, 3, 2, 0], kg_desc={0, 2})

# matmul dtype mode: "bf16" (single pass, ~8-bit mantissa, half the HBM
# bytes), "fp32r" (~11-bit mantissa, 4x PE cost at width<256), "fp32"
# (exact, 4x slower PE)
MODE = "bf16"
C_BF16 = True      # store C as bf16 (halves output traffic)

# pool buffer counts (double/triple buffering)
BUFS_B = 6
BUFS_O = 6
BUFS_PS = 8

# drain engine assignment, cycled per tile: copy 0=DVE 1=ACT(scalar.copy),
# store 0=ACT ring 1=SP ring
DRAIN_COPY = [0]
DRAIN_STORE = [0, 1]

# load masked chunks as one full rectangle (below-diagonal region of the
# packed B is naturally zero) instead of 4 staircase DMAs
MASKED_FULL_LOAD = False

# split A groups with at least this many tiles into two DMAs so early
# matmuls of the chunk wait on half the bytes
A_SPLIT = 24


def build_nc(mode=MODE, rep=1, variant="full"):
    """rep>1 repeats the whole compute (for dispatch-overhead-cancelling
    timing).  variant: "full" | "nomm" (DMAs only) | "nodma" (matmuls
    only)."""
    if (mode, rep, variant) in _nc_cache:
        return _nc_cache[(mode, rep, variant)]
    dt_in = {
        "bf16": mybir.dt.bfloat16,
        "fp32r": mybir.dt.float32r,
        "fp32": mybir.dt.float32,
    }[mode]
    dt_c = mybir.dt.bfloat16 if C_BF16 else mybir.dt.float32

    nc = bacc.Bacc("TRN2", target_bir_lowering=False, debug=False,
                   num_devices=NCORES)
    # partition-major packed layouts (see pack_inputs): per-partition data
    # is contiguous so every DMA is 128 descriptors of large runs.
    # Apack row = p(k-within-tile), col = t*P + m  (first-use tile order)
    a_dram = nc.dram_tensor("Apack", [P, ATOT * P], dt_in,
                            kind="ExternalInput").ap()
    # B row = m*P + p, col = k*QW + n   (core's q column = 2m + s)
    b_dram = nc.dram_tensor("B", [NM * P, NKT * QW], dt_in,
                            kind="ExternalInput").ap()
    c_dram = nc.dram_tensor("Cout", [NT * P, QW], dt_c,
                            kind="ExternalOutput").ap()

    with tile.TileContext(nc) as tc:
        with ExitStack() as ctx:
            apool = ctx.enter_context(tc.tile_pool(name="apool", bufs=1))
            bpool = ctx.enter_context(tc.tile_pool(name="bpool", bufs=BUFS_B))
            opool = ctx.enter_context(tc.tile_pool(name="opool", bufs=BUFS_O))
            pspool = ctx.enter_context(
                tc.tile_pool(name="pspool", bufs=BUFS_PS, space="PSUM"))

            do_bdma = variant in ("full", "nomm")
            do_mm = variant in ("full", "nodma")
            do_copy = variant in ("full", "nomm", "nodma")
            do_store = variant in ("full", "nomm", "nodma")

            a_sb = apool.tile([P, ATOT, P], dt_in)

            def _load_a_group(g):
                t0, t1 = AGROUPS[g]
                if t0 == t1:
                    return
                cuts = [t0, t1]
                if t1 - t0 >= A_SPLIT:
                    cuts = [t0, (t0 + t1) // 2, t1]
                for lo, hi in zip(cuts, cuts[1:]):
                    nc.sync.dma_start(
                        a_sb[:, lo:hi, :],
                        a_dram[:, lo * P:hi * P].rearrange(
                            "p (t m) -> p t m", m=P))

            def _load_chunk(bt, m, kg, ring=None):
                ring = ring or nc.sync
                if kg == 2 * m + 1 and not MASKED_FULL_LOAD:
                    # masked chunk: diagonal for s=1, all-zero for s=0 -
                    # per k-row load only cols >= 128i (the rest of the
                    # tile is stale and masked out of the matmuls)
                    for i in range(4):
                        col = (4 * kg + i) * QW + 128 * i
                        ring.dma_start(
                            bt[:, i, 128 * i:],
                            b_dram[m * P:(m + 1) * P,
                                   col:col + QW - 128 * i])
                else:
                    ring.dma_start(
                        bt[:],
                        b_dram[m * P:(m + 1) * P,
                               4 * kg * QW:(4 * kg + 4) * QW].rearrange(
                                   "p (ko n) -> p ko n", ko=4))

            def _emit_out(m, j, nj, psums):
                # copy psum j to SBUF (dtype convert) and store the tile
                drain = m in DRAIN_MS
                t = PAIRS.index((j, m))
                ot = opool.tile([P, QW], dt_c, tag="ot")
                if do_mm:
                    # in the drain, optionally spread copies over DVE+ACT
                    # so the tail isn't a serial DVE chain (GPSIMD cannot
                    # read PSUM)
                    if drain and DRAIN_COPY[nj % len(DRAIN_COPY)]:
                        nc.scalar.copy(ot[:], psums[j][:])
                    else:
                        nc.vector.tensor_copy(ot[:], psums[j][:])
                else:
                    src = a_sb[:, 4 * j:4 * j + 4, :]
                    if dt_in == mybir.dt.float32r:
                        src = src.bitcast(mybir.dt.float32)
                    nc.vector.tensor_copy(
                        ot[:].rearrange("p (a b) -> p a b", a=4), src)
                if do_store:
                    # ACT HWDGE ring keeps compute-gated stores out of the
                    # B-stream's SP FIFO; in the drain the B stream is
                    # done, so SP is free too
                    ring = (nc.sync if drain
                            and DRAIN_STORE[nj % len(DRAIN_STORE)]
                            else nc.scalar)
                    ring.dma_start(c_dram[t * P:(t + 1) * P, :], ot[:])

            bt_fixed = None
            for _r in range(rep):
                mpsums = {}
                mdone = {m: [] for m in M_ORDER}
                last_pos = {m: max(p for p, (mm, _) in enumerate(CHUNK_SEQ)
                                   if mm == m) for m in M_ORDER}
                for g, (m, kg) in enumerate(CHUNK_SEQ):
                    act = list(range(2 * m + 2))
                    if do_mm and m not in mpsums:
                        mpsums[m] = {
                            j: pspool.tile([P, QW], mybir.dt.float32,
                                           tag="ps", name=f"ps_{_r}_{m}_{j}")
                            for j in act
                        }
                    psums = mpsums.get(m, {})
                    if _r == 0:
                        _load_a_group(g)
                    if do_bdma:
                        bt = bpool.tile([P, 4, QW], dt_in, tag="bt")
                        _load_chunk(bt, m, kg)
                    elif do_mm:
                        if bt_fixed is None:
                            bt_fixed = bpool.tile([P, 4, QW], dt_in,
                                                  tag="bt", name="bt_fixed")
                            _load_chunk(bt_fixed, 0, 0)
                        bt = bt_fixed
                    if do_mm and (do_bdma or bt_fixed is not None):
                        masked = kg == 2 * m + 1
                        # j-major within the masked chunk so psum
                        # stops/starts stagger
                        for j in (act if masked else [None]):
                            for i in range(4):
                                k = 4 * kg + i
                                for jj in ([j] if masked else act):
                                    if k < 4 * jj:
                                        continue
                                    c0 = 128 * i if masked else 0
                                    desc = m in KG_DESC
                                    first = (k == 8 * m + 4 if desc
                                             else k == 4 * jj)
                                    last = (k == 4 * jj + 3 if desc
                                            else k == 8 * m + 7)
                                    nc.tensor.matmul(
                                        psums[jj][:, c0:],
                                        a_sb[:, ATIDX[(jj, k)], :],
                                        bt[:, i, c0:],
                                        start=first, stop=last)
                    if not (do_copy or do_store):
                        continue
                    if m in KG_DESC and do_mm:
                        # chunk kg completes exactly psum j = kg
                        _emit_out(m, kg, len(mdone[m]), psums)
                        mdone[m].append(kg)
                    if g == last_pos[m]:
                        for nj, j in enumerate(act):
                            if j in mdone[m]:
                                continue
                            if do_mm and not (do_bdma
                                              or bt_fixed is not None):
                                continue
                            _emit_out(m, j, nj, psums)
    nc.compile()
    _nc_cache[(mode, rep, variant)] = nc
    return nc


def pack_inputs(A, B, mode=MODE):
    """Build per-core in_maps (partition-major packed layouts)."""
    import ml_dtypes
    A = np.ascontiguousarray(np.asarray(A, dtype=np.float32))
    B = np.ascontiguousarray(np.asarray(B, dtype=np.float32))
    dt_np = {
        "bf16": ml_dtypes.bfloat16,
        "fp32r": np.float32,
        "fp32": np.float32,
    }[mode]

    # B[128k+p, 512q+n] -> per col-group s: Bp[m, p, k, n] with q = 2m+s
    b_packs = []
    b4 = B.reshape(NKT, P, GS * NM, QW)
    for s in range(GS):
        qsel = [2 * m + s for m in range(NM)]
        b_packs.append(np.ascontiguousarray(
            b4[:, :, qsel, :].transpose(2, 1, 0, 3).astype(dt_np)
        ).reshape(NM * P, NKT * QW))

    in_maps = []
    for c in range(NCORES):
        r, s = c % GR, c // GR
        ap = np.zeros((ATOT, P, P), np.float32)
        for (j, k), t in ATIDX.items():
            b = GR * j + r
            if k >= b:
                ap[t] = A[P * b:P * b + P, P * k:P * k + P].T
        # [t, p, m] -> [p, t, m] -> [P, ATOT*P]
        apk = np.ascontiguousarray(
            ap.astype(dt_np).transpose(1, 0, 2)).reshape(P, ATOT * P)
        in_maps.append({"Apack": apk, "B": b_packs[s]})
    return in_maps


def unpack_output(results):
    C = np.zeros((N, N), np.float32)
    for c, r in enumerate(results):
        rr, s = c % GR, c // GR
        co = np.asarray(r["Cout"]).astype(np.float32).reshape(NT, P, QW)
        for t, (j, m) in enumerate(PAIRS):
            b = GR * j + rr
            q = 2 * m + s
            if P * b >= QW * (q + 1):
                continue               # fully below-diagonal tile: zeros
            C[P * b:P * b + P, QW * q:QW * q + QW] = co[t]
    return C


def kernel(A, B):
    nc = build_nc(MODE)
    in_maps = pack_inputs(A, B, MODE)
    res = bass_utils.run_bass_kernel_spmd(
        nc, in_maps, core_ids=list(range(NCORES)), trace=False)
    return unpack_output(res.results)


# revision 56
# speedup vs baseline: 2.0689x; 1.0065x over previous
"""Trainium2 Bass kernel: C = triu(A @ B), A/B upper-triangular 4096x4096 fp32.

Strategy (2D: 4 row-groups x 2 col-groups, SPMD single program, bf16):
  * Core c -> (r = c%4, s = c//4). Core owns row blocks {4j + r, j=0..7}
    (8 slots of 128 rows) and output column tiles q in {2m+s, m=0..3}
    (4 slots of 512 cols).  vs. the 1D row-parallel layout this halves
    the per-core B stream (only its own q columns) at the cost of 2x A.
  * One uniform schedule for all cores: for m-slot and chunk kg (4
    k-tiles of 128), accumulate k in [4j, 8m+7] into psum[j] for every
    active slot j <= 2m+1.  Per-core variation lives entirely in the
    DATA: A^T tiles with k < own-block are zero-filled by the host, and
    B's below-diagonal blocks are zero in the source matrix, so padded
    matmuls contribute exact zeros.  Since A and B are upper-triangular
    the lower triangle of C comes out exactly 0 - no masking needed.
  * bf16 inputs (rel-err ~2e-3 vs 2e-2 budget) halve HBM bytes; C is
    stored bf16 too (rel-err ~4e-3).  Per-core HBM traffic ~16.6 MB,
    PE ~128k columns: balanced at ~53us each.
  * The masked chunk of each m (kg = 2m+1) is the diagonal chunk for
    s=1 cores and all-zero for s=0 cores: loaded with the triangle
    pattern (cols >= 128i of k-row i) and matmul'd width-masked -
    correct for both.  A^T is packed in exact first-use order of the
    emission schedule and streamed just-in-time, one group per chunk.
  * Schedule (model-swept): m order [1,3,2,0] - tiny m=0 last so the
    copy/store drain tail is short; kg descending inside m=2 and m=0,
    which completes psum j exactly at chunk kg=j and spreads the drain.
"""

import numpy as np
from contextlib import ExitStack

import concourse.mybir as mybir
import concourse.tile as tile
from concourse import bacc, bass_utils

N = 4096
P = 128
NCORES = 8
GR = 4             # row groups (cores per column group)
GS = 2             # column groups
NJ = 8             # row-block slots per core (32 blocks / GR)
NM = 4             # 512-wide output column slots per core (8 q-tiles / GS)
QW = 512
NKT = 32           # 128-wide k tiles

# (slot, m) pairs the program computes/writes, in emission order
PAIRS = [(j, m) for m in range(NM) for j in range(2 * m + 2)]
NT = len(PAIRS)    # 20 output tiles of 128x512 per core


def _kgs(m):
    """Chunk order within an m-slot.  Descending puts the k-tiles shared
    by many slots first (more PE work per DMA byte early) and completes
    psum j exactly at chunk kg=j, spreading the copy/store drain across
    the whole m instead of bunching it at the end."""
    r = range(2 * m + 2)
    return list(reversed(r)) if m in KG_DESC else list(r)


def _a_layout(seq):
    """A^T tiles in exact first-use order of the emission schedule; group
    g holds the tiles first needed by chunk g, so A streams just-in-time
    interleaved with the B chunks."""
    idx = {}
    groups = []
    t = 0
    for m, kg in seq:
        t0 = t
        for i in range(4):
            k = 4 * kg + i
            for j in range(2 * m + 2):
                if k >= 4 * j and (j, k) not in idx:
                    idx[(j, k)] = t
                    t += 1
        groups.append((t0, t))
    return idx, groups


def set_order(order, kg_desc=None, seq=None):
    """Set the m emission order and recompute the derived layout.  seq
    overrides the chunk sequence (must preserve within-m kg order of
    _kgs and contain every (m, kg) exactly once)."""
    global M_ORDER, ATIDX, AGROUPS, ATOT, DRAIN_MS, KG_DESC, CHUNK_SEQ
    if kg_desc is not None:
        KG_DESC = set(kg_desc)
    M_ORDER = list(order)
    CHUNK_SEQ = (list(seq) if seq is not None else
                 [(m, kg) for m in M_ORDER for kg in _kgs(m)])
    ATIDX, AGROUPS = _a_layout(CHUNK_SEQ)
    ATOT = max(t1 for _, t1 in AGROUPS)    # 144 packed A tiles per core
    DRAIN_MS = set(M_ORDER[-2:])
    _nc_cache.clear()


_nc_cache = {}
KG_DESC = set()
# m emission order: big-PE m=3 late so DMA prefetch runs ahead, tiny
# m=0 (2 output tiles) last so the store-drain tail is short; kg
# descending inside m=2 and m=0 (model-swept optimum)
set_order([1, 3, 2, 0], kg_desc={0, 2})

# matmul dtype mode: "bf16" (single pass, ~8-bit mantissa, half the HBM
# bytes), "fp32r" (~11-bit mantissa, 4x PE cost at width<256), "fp32"
# (exact, 4x slower PE)
MODE = "bf16"
C_BF16 = True      # store C as bf16 (halves output traffic)

# pool buffer counts (double/triple buffering)
BUFS_B = 6
BUFS_O = 6
BUFS_PS = 8

# drain engine assignment, cycled per tile: copy 0=DVE 1=ACT(scalar.copy),
# store 0=ACT ring 1=SP ring
DRAIN_COPY = [0]
DRAIN_STORE = [0, 1]

# load masked chunks as one full rectangle (below-diagonal region of the
# packed B is naturally zero) instead of 4 staircase DMAs
MASKED_FULL_LOAD = False

# split A groups with at least this many tiles into two DMAs so early
# matmuls of the chunk wait on half the bytes
A_SPLIT = 24

# chunk-sequence positions whose full B chunk loads as two halves
# (model-swept local optimum for the supply-bound early window)
B_SPLIT_POS = {2, 5}


def build_nc(mode=MODE, rep=1, variant="full"):
    """rep>1 repeats the whole compute (for dispatch-overhead-cancelling
    timing).  variant: "full" | "nomm" (DMAs only) | "nodma" (matmuls
    only)."""
    if (mode, rep, variant) in _nc_cache:
        return _nc_cache[(mode, rep, variant)]
    dt_in = {
        "bf16": mybir.dt.bfloat16,
        "fp32r": mybir.dt.float32r,
        "fp32": mybir.dt.float32,
    }[mode]
    dt_c = mybir.dt.bfloat16 if C_BF16 else mybir.dt.float32

    nc = bacc.Bacc("TRN2", target_bir_lowering=False, debug=False,
                   num_devices=NCORES)
    # partition-major packed layouts (see pack_inputs): per-partition data
    # is contiguous so every DMA is 128 descriptors of large runs.
    # Apack row = p(k-within-tile), col = t*P + m  (first-use tile order)
    a_dram = nc.dram_tensor("Apack", [P, ATOT * P], dt_in,
                            kind="ExternalInput").ap()
    # B row = m*P + p, col = k*QW + n   (core's q column = 2m + s)
    b_dram = nc.dram_tensor("B", [NM * P, NKT * QW], dt_in,
                            kind="ExternalInput").ap()
    c_dram = nc.dram_tensor("Cout", [NT * P, QW], dt_c,
                            kind="ExternalOutput").ap()

    with tile.TileContext(nc) as tc:
        with ExitStack() as ctx:
            apool = ctx.enter_context(tc.tile_pool(name="apool", bufs=1))
            bpool = ctx.enter_context(tc.tile_pool(name="bpool", bufs=BUFS_B))
            opool = ctx.enter_context(tc.tile_pool(name="opool", bufs=BUFS_O))
            pspool = ctx.enter_context(
                tc.tile_pool(name="pspool", bufs=BUFS_PS, space="PSUM"))

            do_bdma = variant in ("full", "nomm")
            do_mm = variant in ("full", "nodma")
            do_copy = variant in ("full", "nomm", "nodma")
            do_store = variant in ("full", "nomm", "nodma")

            a_sb = apool.tile([P, ATOT, P], dt_in)

            def _load_a_group(g):
                t0, t1 = AGROUPS[g]
                if t0 == t1:
                    return
                cuts = [t0, t1]
                if t1 - t0 >= A_SPLIT:
                    cuts = [t0, (t0 + t1) // 2, t1]
                for lo, hi in zip(cuts, cuts[1:]):
                    nc.sync.dma_start(
                        a_sb[:, lo:hi, :],
                        a_dram[:, lo * P:hi * P].rearrange(
                            "p (t m) -> p t m", m=P))

            def _load_chunk(bt, m, kg, ring=None, split=False):
                ring = ring or nc.sync
                if kg == 2 * m + 1 and not MASKED_FULL_LOAD:
                    # masked chunk: diagonal for s=1, all-zero for s=0 -
                    # per k-row load only cols >= 128i (the rest of the
                    # tile is stale and masked out of the matmuls)
                    for i in range(4):
                        col = (4 * kg + i) * QW + 128 * i
                        ring.dma_start(
                            bt[:, i, 128 * i:],
                            b_dram[m * P:(m + 1) * P,
                                   col:col + QW - 128 * i])
                elif split:
                    # two half-chunk DMAs so early-k matmuls gate on half
                    # the bytes (supply-critical windows only)
                    for h in (0, 2):
                        col = (4 * kg + h) * QW
                        ring.dma_start(
                            bt[:, h:h + 2, :],
                            b_dram[m * P:(m + 1) * P,
                                   col:col + 2 * QW].rearrange(
                                       "p (ko n) -> p ko n", ko=2))
                else:
                    ring.dma_start(
                        bt[:],
                        b_dram[m * P:(m + 1) * P,
                               4 * kg * QW:(4 * kg + 4) * QW].rearrange(
                                   "p (ko n) -> p ko n", ko=4))

            def _emit_out(m, j, nj, psums):
                # copy psum j to SBUF (dtype convert) and store the tile
                drain = m in DRAIN_MS
                t = PAIRS.index((j, m))
                ot = opool.tile([P, QW], dt_c, tag="ot")
                if do_mm:
                    # in the drain, optionally spread copies over DVE+ACT
                    # so the tail isn't a serial DVE chain (GPSIMD cannot
                    # read PSUM)
                    if drain and DRAIN_COPY[nj % len(DRAIN_COPY)]:
                        nc.scalar.copy(ot[:], psums[j][:])
                    else:
                        nc.vector.tensor_copy(ot[:], psums[j][:])
                else:
                    src = a_sb[:, 4 * j:4 * j + 4, :]
                    if dt_in == mybir.dt.float32r:
                        src = src.bitcast(mybir.dt.float32)
                    nc.vector.tensor_copy(
                        ot[:].rearrange("p (a b) -> p a b", a=4), src)
                if do_store:
                    # ACT HWDGE ring keeps compute-gated stores out of the
                    # B-stream's SP FIFO; in the drain the B stream is
                    # done, so SP is free too
                    ring = (nc.sync if drain
                            and DRAIN_STORE[nj % len(DRAIN_STORE)]
                            else nc.scalar)
                    ring.dma_start(c_dram[t * P:(t + 1) * P, :], ot[:])

            bt_fixed = None
            for _r in range(rep):
                mpsums = {}
                mdone = {m: [] for m in M_ORDER}
                last_pos = {m: max(p for p, (mm, _) in enumerate(CHUNK_SEQ)
                                   if mm == m) for m in M_ORDER}
                for g, (m, kg) in enumerate(CHUNK_SEQ):
                    act = list(range(2 * m + 2))
                    if do_mm and m not in mpsums:
                        mpsums[m] = {
                            j: pspool.tile([P, QW], mybir.dt.float32,
                                           tag="ps", name=f"ps_{_r}_{m}_{j}")
                            for j in act
                        }
                    psums = mpsums.get(m, {})
                    if _r == 0:
                        _load_a_group(g)
                    if do_bdma:
                        bt = bpool.tile([P, 4, QW], dt_in, tag="bt")
                        _load_chunk(bt, m, kg, split=g in B_SPLIT_POS)
                    elif do_mm:
                        if bt_fixed is None:
                            bt_fixed = bpool.tile([P, 4, QW], dt_in,
                                                  tag="bt", name="bt_fixed")
                            _load_chunk(bt_fixed, 0, 0)
                        bt = bt_fixed
                    if do_mm and (do_bdma or bt_fixed is not None):
                        masked = kg == 2 * m + 1
                        # j-major within the masked chunk so psum
                        # stops/starts stagger
                        for j in (act if masked else [None]):
                            for i in range(4):
                                k = 4 * kg + i
                                for jj in ([j] if masked else act):
                                    if k < 4 * jj:
                                        continue
                                    c0 = 128 * i if masked else 0
                                    desc = m in KG_DESC
                                    first = (k == 8 * m + 4 if desc
                                             else k == 4 * jj)
                                    last = (k == 4 * jj + 3 if desc
                                            else k == 8 * m + 7)
                                    nc.tensor.matmul(
                                        psums[jj][:, c0:],
                                        a_sb[:, ATIDX[(jj, k)], :],
                                        bt[:, i, c0:],
                                        start=first, stop=last)
                    if not (do_copy or do_store):
                        continue
                    if m in KG_DESC and do_mm:
                        # chunk kg completes exactly psum j = kg
                        _emit_out(m, kg, len(mdone[m]), psums)
                        mdone[m].append(kg)
                    if g == last_pos[m]:
                        for nj, j in enumerate(act):
                            if j in mdone[m]:
                                continue
                            if do_mm and not (do_bdma
                                              or bt_fixed is not None):
                                continue
                            _emit_out(m, j, nj, psums)
    nc.compile()
    _nc_cache[(mode, rep, variant)] = nc
    return nc


def pack_inputs(A, B, mode=MODE):
    """Build per-core in_maps (partition-major packed layouts)."""
    import ml_dtypes
    A = np.ascontiguousarray(np.asarray(A, dtype=np.float32))
    B = np.ascontiguousarray(np.asarray(B, dtype=np.float32))
    dt_np = {
        "bf16": ml_dtypes.bfloat16,
        "fp32r": np.float32,
        "fp32": np.float32,
    }[mode]

    # B[128k+p, 512q+n] -> per col-group s: Bp[m, p, k, n] with q = 2m+s
    b_packs = []
    b4 = B.reshape(NKT, P, GS * NM, QW)
    for s in range(GS):
        qsel = [2 * m + s for m in range(NM)]
        b_packs.append(np.ascontiguousarray(
            b4[:, :, qsel, :].transpose(2, 1, 0, 3).astype(dt_np)
        ).reshape(NM * P, NKT * QW))

    in_maps = []
    for c in range(NCORES):
        r, s = c % GR, c // GR
        ap = np.zeros((ATOT, P, P), np.float32)
        for (j, k), t in ATIDX.items():
            b = GR * j + r
            if k >= b:
                ap[t] = A[P * b:P * b + P, P * k:P * k + P].T
        # [t, p, m] -> [p, t, m] -> [P, ATOT*P]
        apk = np.ascontiguousarray(
            ap.astype(dt_np).transpose(1, 0, 2)).reshape(P, ATOT * P)
        in_maps.append({"Apack": apk, "B": b_packs[s]})
    return in_maps


def unpack_output(results):
    C = np.zeros((N, N), np.float32)
    for c, r in enumerate(results):
        rr, s = c % GR, c // GR
        co = np.asarray(r["Cout"]).astype(np.float32).reshape(NT, P, QW)
        for t, (j, m) in enumerate(PAIRS):
            b = GR * j + rr
            q = 2 * m + s
            if P * b >= QW * (q + 1):
                continue               # fully below-diagonal tile: zeros
            C[P * b:P * b + P, QW * q:QW * q + QW] = co[t]
    return C


def kernel(A, B):
    nc = build_nc(MODE)
    in_maps = pack_inputs(A, B, MODE)
    res = bass_utils.run_bass_kernel_spmd(
        nc, in_maps, core_ids=list(range(NCORES)), trace=False)
    return unpack_output(res.results)
